# revision 8
# baseline (speedup 1.0000x reference)
"""Bass/Trainium2 kernel for nn_GreedyMatcher: batched PDHG LP solver.

Reference computation (per batch sample b):
    B = X.reshape(bs, 128); Wb = broadcast(W)
    x0 = y0 = 0, xbar0 = 0
    repeat 100x:
        y   = relu(y + sigma*(xbar @ S.T - B))
        x'  = relu(x + tau*(W - y @ S))
        xbar = 2x' - x ; x = x'
    return x  [bs, 2048]

Strategy: pure data parallel over batch (256 -> 32 per core, 8 cores).
Per-core state is kept struct-major in SBUF: Xsb[p, 32*m + b] = x[b, 128*m + p]
so both matmuls per iteration run with K=128 chunks on the tensor engine.
The extrapolation xbar is never materialized: with V_t = S @ x_t^T,
S @ xbar_t^T = 2 V_t - V_{t-1}, and the dual update folds into a single
carried tensor g_t = y_t - sigma*V_{t-1} - sigma*B^T:
    v      = (2 sigma V_t) + g_t          # psum + g
    y_.    = relu(v)
    g_{t+1}= y_. - sigma*V_t - sigma*B^T
    x_{t+1}= relu(x_t + tau - tau*(S^T y_.))   (W == ones fast path)
"""

import sys
import os

sys.path.insert(0, "/opt/trn_rl_repo")

import numpy as np

N_CORES = 8
BATCH = 256
BS = BATCH // N_CORES  # 32 per core
N_HOS = 8
N_TYPES = 16
M_CONS = N_HOS * N_TYPES  # 128 constraints
N_STRUCTS = 2048
N_CHUNKS = N_STRUCTS // 128  # 16
N_ITERS = 100
N_STREAMS = 2

_CACHE = {}


def _spec_norm_f32(S: np.ndarray) -> np.float32:
    """Mimic reference._spec_norm in float32 numpy."""
    S = S.astype(np.float32)
    v = np.ones((S.shape[1],), np.float32)
    v = v / np.float32(np.linalg.norm(v))
    for _ in range(30):
        u = S @ v
        u = u / (np.float32(np.linalg.norm(u)) + np.float32(1e-12))
        v = S.T @ u
        v = v / (np.float32(np.linalg.norm(v)) + np.float32(1e-12))
    return np.float32(np.linalg.norm(S @ v))


def _get_compiled(tau: float, sigma: float, mm_dtype_name: str, w_is_ones: bool):
    key = (round(float(tau), 12), round(float(sigma), 12), mm_dtype_name, w_is_ones)
    if key in _CACHE:
        return _CACHE[key]
    nc = _build_real(mm_dtype_name, w_is_ones, float(tau), float(sigma))
    nc.compile()
    _CACHE[key] = nc
    return nc


def _build_real(mm_dtype_name: str, w_is_ones: bool, tau: float, sigma: float):
    import concourse.bacc as bacc
    import concourse.tile as tile
    import concourse.mybir as mybir
    from contextlib import ExitStack

    f32 = mybir.dt.float32
    mmdt = getattr(mybir.dt, mm_dtype_name)
    ALU = mybir.AluOpType
    ACT = mybir.ActivationFunctionType

    nc = bacc.Bacc(None, target_bir_lowering=False)

    STs_d = nc.dram_tensor("STs", [128, N_STRUCTS], mmdt, kind="ExternalInput")
    Ss_d = nc.dram_tensor("Ss", [128, N_STRUCTS], mmdt, kind="ExternalInput")
    Bs_d = nc.dram_tensor("Bs", [128, BS], f32, kind="ExternalInput")
    if not w_is_ones:
        TW_d = nc.dram_tensor("TW", [128, N_CHUNKS * BS], f32, kind="ExternalInput")
    XO_d = nc.dram_tensor("XOUT", [128, N_CHUNKS * BS], f32, kind="ExternalOutput")

    NS = N_STREAMS
    HB = BS // NS          # samples per stream
    SFD = N_CHUNKS * HB    # free dim of per-stream x state

    with tile.TileContext(nc) as tc:
        with ExitStack() as ctx:
            const = ctx.enter_context(tc.tile_pool(name="const", bufs=1))
            state = ctx.enter_context(tc.tile_pool(name="state", bufs=1))
            tmp = ctx.enter_context(tc.tile_pool(name="tmp", bufs=3))
            psum = ctx.enter_context(tc.tile_pool(name="psum", bufs=2, space="PSUM"))

            STs = const.tile([128, N_STRUCTS], mmdt, tag="STs")
            Ss = const.tile([128, N_STRUCTS], mmdt, tag="Ss")
            Bs = const.tile([128, BS], f32, tag="Bs")
            nc.sync.dma_start(STs[:], STs_d[:])
            nc.sync.dma_start(Ss[:], Ss_d[:])
            nc.sync.dma_start(Bs[:], Bs_d[:])
            if not w_is_ones:
                TW = const.tile([128, N_CHUNKS * BS], f32, tag="TW")
                nc.sync.dma_start(TW[:], TW_d[:])

            zb = const.tile([128, 1], f32, tag="zb")
            taub = const.tile([128, 1], f32, tag="taub")
            nc.gpsimd.memset(zb[:], 0.0)
            nc.gpsimd.memset(taub[:], float(tau))

            two_sigma = float(2.0 * sigma)
            neg_sigma = float(-sigma)

            # per-stream state
            x32 = [[state.tile([128, SFD], f32, name=f"x32_{s}_{i}", tag=f"x32_{s}_{i}")
                    for i in range(2)] for s in range(NS)]
            x16 = [[state.tile([128, SFD], mmdt, name=f"x16_{s}_{i}", tag=f"x16_{s}_{i}")
                    for i in range(2)] for s in range(NS)]
            gb = [[state.tile([128, HB], f32, name=f"g_{s}_{i}", tag=f"g_{s}_{i}")
                   for i in range(2)] for s in range(NS)]

            for s in range(NS):
                nc.gpsimd.memset(x16[s][0][:], 0.0)
                nc.gpsimd.memset(x32[s][0][:], 0.0)
                nc.vector.tensor_scalar_mul(
                    gb[s][0][:], Bs[:, HB * s : HB * (s + 1)], -1.0
                )

            def dual_phase(s, t):
                x16_cur = x16[s][t % 2]
                pV = psum.tile([128, HB], f32, tag=f"pV_{s}")
                for k in range(N_CHUNKS):
                    nc.tensor.matmul(
                        pV[:],
                        STs[:, 128 * k : 128 * (k + 1)],
                        x16_cur[:, HB * k : HB * (k + 1)],
                        start=(k == 0),
                        stop=(k == N_CHUNKS - 1),
                    )
                v = tmp.tile([128, HB], f32, tag=f"v_{s}", name=f"v_{s}")
                nc.vector.scalar_tensor_tensor(
                    v[:], pV[:], two_sigma, gb[s][t % 2][:], ALU.mult, ALU.add
                )
                y16 = tmp.tile([128, HB], mmdt, tag=f"y16_{s}", name=f"y16_{s}")
                nc.vector.tensor_scalar_max(y16[:], v[:], 0.0)
                # fp32 dual bookkeeping (off critical path)
                y32 = tmp.tile([128, HB], f32, tag=f"y32_{s}", name=f"y32_{s}")
                nc.gpsimd.tensor_scalar_max(y32[:], v[:], 0.0)
                yB = tmp.tile([128, HB], f32, tag=f"yB_{s}", name=f"yB_{s}")
                nc.gpsimd.tensor_sub(yB[:], y32[:], Bs[:, HB * s : HB * (s + 1)])
                nc.vector.scalar_tensor_tensor(
                    gb[s][(t + 1) % 2][:], pV[:], neg_sigma, yB[:], ALU.mult, ALU.add
                )
                return y16

            def primal_phase(s, t, y16):
                x32_cur = x32[s][t % 2]
                pX = psum.tile([128, SFD], f32, tag=f"pX_{s}")
                for m in range(N_CHUNKS):
                    nc.tensor.matmul(
                        pX[:, HB * m : HB * (m + 1)],
                        Ss[:, 128 * m : 128 * (m + 1)],
                        y16[:],
                        start=True,
                        stop=True,
                    )
                e = tmp.tile([128, SFD], f32, tag=f"e_{s}", name=f"e_{s}")
                nc.vector.scalar_tensor_tensor(
                    e[:], pX[:], float(-tau), x32_cur[:], ALU.mult, ALU.add
                )
                if w_is_ones:
                    nc.scalar.activation(
                        x16[s][(t + 1) % 2][:], e[:], ACT.Relu, bias=taub[:]
                    )
                    nc.scalar.activation(
                        x32[s][(t + 1) % 2][:], e[:], ACT.Relu, bias=taub[:]
                    )
                else:
                    # TW is host-arranged so that columns [SFD*s : SFD*(s+1)]
                    # hold tau*W in this stream's (m, b) layout.
                    e2 = tmp.tile([128, SFD], f32, tag=f"e2_{s}", name=f"e2_{s}")
                    nc.vector.tensor_add(e2[:], e[:], TW[:, SFD * s : SFD * (s + 1)])
                    nc.scalar.activation(
                        x16[s][(t + 1) % 2][:], e2[:], ACT.Relu, bias=zb[:]
                    )
                    nc.scalar.activation(
                        x32[s][(t + 1) % 2][:], e2[:], ACT.Relu, bias=zb[:]
                    )

            for t in range(N_ITERS):
                y16s = [None] * NS
                for s in range(NS):
                    y16s[s] = dual_phase(s, t)
                for s in range(NS):
                    primal_phase(s, t, y16s[s])

            for s in range(NS):
                nc.sync.dma_start(
                    XO_d[:, SFD * s : SFD * (s + 1)], x32[s][N_ITERS % 2][:]
                )

    return nc


MM_DTYPE = os.environ.get("GM_MM_DTYPE", "float32")


def kernel_run(X, S, W, batch_size, trace=False, tmpdir=None):
    from concourse.bass_utils import run_bass_kernel_spmd

    X = np.asarray(X, np.float32)
    S = np.asarray(S, np.float32)
    W = np.asarray(W, np.float32)
    bs = int(batch_size)
    assert bs == BATCH and X.shape == (BATCH, N_HOS, N_TYPES)
    assert S.shape == (M_CONS, N_STRUCTS)

    L = _spec_norm_f32(S)
    sigma = np.float32(0.9) / L
    tau = np.float32(0.9) / L

    B = X.reshape(BATCH, M_CONS)
    w_is_ones = bool(np.all(W == 1.0))

    np_mmdt = {"float32": np.float32, "float16": np.float16}[MM_DTYPE]
    # STs[p, 128k+j] = S[j, 128k+p]  (exact 0/1 in fp16)
    STs = (
        S.T.reshape(N_CHUNKS, 128, 128)
        .transpose(1, 0, 2)
        .reshape(128, N_STRUCTS)
        .astype(np_mmdt)
    )
    Ss = S.astype(np_mmdt)

    in_maps = []
    for c in range(N_CORES):
        Bs_c = (sigma * B[BS * c : BS * (c + 1), :]).T.astype(np.float32)
        m = {"STs": STs, "Ss": Ss, "Bs": np.ascontiguousarray(Bs_c)}
        if not w_is_ones:
            HB = BS // N_STREAMS
            TW_c = np.broadcast_to(
                (tau * W).reshape(N_CHUNKS, 128, 1), (N_CHUNKS, 128, HB)
            )
            one_stream = (
                TW_c.transpose(1, 0, 2).reshape(128, N_CHUNKS * HB).astype(np.float32)
            )
            m["TW"] = np.ascontiguousarray(np.tile(one_stream, (1, N_STREAMS)))
        in_maps.append(m)

    nc = _get_compiled(float(tau), float(sigma), MM_DTYPE, w_is_ones)
    res = run_bass_kernel_spmd(
        nc, in_maps, list(range(N_CORES)), trace=trace, tmpdir=tmpdir
    )

    out = np.empty((BATCH, N_STRUCTS), np.float32)
    HB = BS // N_STREAMS
    SFD = N_CHUNKS * HB
    for c in range(N_CORES):
        O = res.results[c]["XOUT"]  # [128, N_CHUNKS*BS]
        for s in range(N_STREAMS):
            Os = O[:, SFD * s : SFD * (s + 1)]
            out[BS * c + HB * s : BS * c + HB * (s + 1), :] = (
                Os.reshape(128, N_CHUNKS, HB).transpose(2, 1, 0).reshape(HB, N_STRUCTS)
            )
    return out, res


def kernel(**inputs):
    out, _ = kernel_run(
        inputs["X"], inputs["S"], inputs["W"], inputs["batch_size"], trace=False
    )
    return out
